# revision 10
# baseline (speedup 1.0000x reference)
"""Distributed Trainium2 Bass kernel for a single attention head.

Problem (hardcoded): q,k,v [4, 4096, 1024] f32, Wq/Wk/Wv [1024, 64] f32,
attn_mask [4096, 4096] bool (True = keep).  out[b] = softmax(mask(q Wq (k Wk)^T) / 8) (v Wv).

Sharding: 8 cores; core c -> batch c//2, parity par = c%2.  The k/v rows of
the batch are split by 128-row k-tile parity: core par owns global k-tiles
{2i+par}.  Each core computes, for every 512-row query chunk j, the partial
(unnormalized) attention output sum_k exp(s)*v and the partial denominator
over ITS k-tiles only.  The host sums the two cores' partials and normalizes
(flash-attention style additive combine; no on-device collectives needed --
pair collectives have a ~7-20us latency floor, worse than the ~7us of PE
work they would dedup).

On-device layout / scheduling tricks (v2 -- tensor-engine-bound rework):
- All inputs are host-pre-tiled into the exact [128-partition, d-tile, col]
  SBUF layouts so every DMA is fully contiguous; every staged x chunk gets
  its own SBUF buffer so all input DMAs issue up front.
- q chunks are DMAed FIRST (q0..q3 in window 0) so projections (the only
  PE work available at the start) begin ~5.5us in, right after the ~3.4us
  engine-preamble barrier + first transfers; k/v stream behind.
- A burst of dummy matmuls on a memset tile warms the PE HAM clock gate
  (1.2 -> 2.4 GHz) during the otherwise-idle DMA ramp, so real matmuls
  start at the warm 2.4 GHz issue rate (214ns per 512-col matmul).
- Wq / Wk are duplicated column-wise in the packed weight so the projections
  produce qh / kh replicated in both partition halves.  Score matmuls have
  K=64; even/odd local k-tiles are stored in partition halves 0-63 / 64-127,
  so each beat's two score matmuls land in disjoint PE row-groups and
  overlap partially in the array (~385ns/pair vs 428ns serial).
- Causal diagonal trim: for score tiles whose leading query columns are
  fully masked, the score matmul streams only the valid column suffix and
  packs beats' valid spans contiguously in PSUM; exp() then runs on the
  packed (shorter) span and the PV matmul consumes the same packed slice,
  writing the matching suffix of the output accumulator.  The first tile of
  each chunk is never trimmed so the PV accumulation's start=True pass
  covers every output column.
- exp() runs on the packed beat span (<=1024 cols) to amortize the ~293ns
  ACT call overhead; mask multiplies for partially-kept 512x128 blocks run
  on the otherwise-idle GPSIMD engine (0/1 tiles from a tiny deduplicated
  table), keeping DVE free for the projection-finisher casts.
- Output partials are written as bf16 and drained via the gpsimd DMA queue
  while input DMAs are still issuing (the sync sequencer is in-order); the
  final window's drains go through the faster sync HWDGE queue instead,
  since by then all input DMAs have issued.
"""

import os
import sys

sys.path.insert(0, "/opt/trn_rl_repo")

import numpy as np
import ml_dtypes

import concourse.bass as bass
import concourse.mybir as mybir
import concourse.tile as tile
from concourse import bacc
from concourse.bass_utils import run_bass_kernel_spmd
from concourse.masks import make_identity

F32 = mybir.dt.float32
BF16 = mybir.dt.bfloat16
FP8 = mybir.dt.float8e4
BF16_NP = ml_dtypes.bfloat16
FP8_NP = ml_dtypes.float8_e4m3

N_CORES = 8
B, T, D, H = 4, 4096, 1024, 64
P = 128                      # partitions / k-tile rows
QC = 512                     # query chunk width
NJ = T // QC                 # 8 query chunks
GT = T // P                  # 32 global k-tiles
LT = GT // 2                 # 16 local (per-parity) k-tiles
D_TILES = D // P             # 8
KVW = 512                    # k/v projection chunk width (4 local tiles)
NKV = LT * P // KVW          # 4 kv chunks / emission blocks
# DMA window contents: q front-loaded so the PE has projection work as
# early as possible; k/v stream behind in tile order.
WIN_DMA = [
    [("wqv",), ("q", 0), ("wk",), ("k", 0), ("v", 0), ("q", 1), ("q", 2),
     ("q", 3)],
    [("masks",), ("q", 4), ("q", 5), ("k", 1), ("v", 1)],
    [("q", 6), ("q", 7), ("k", 2), ("v", 2)],
    [("k", 3), ("v", 3)],
    [],
]
NWIN = len(WIN_DMA)
KVWIN = [0, 1, 2, 3]             # window whose thunks project kv chunk c
QWIN = [0, 0, 0, 0, 1, 1, 2, 2]  # window whose thunks project q chunk j
N_WARM = 5                       # HAM warm-up matmuls

LAST_RESULT = None           # test harness reads exec_time_ns from here
_CACHE = {}


def _schedule(mask):
    """Per query chunk j: the list of local k-tile indices this parity pair
    processes (compile-time), per entry the mask-table slot to multiply
    with (None = block fully kept for both parities), and per entry the
    column trim offset (leading fully-masked query columns, same for both
    parities only if... computed per-parity at emission; here we store per
    (j, pos, par))."""
    m = mask.reshape(NJ, QC, GT, P)
    blk_any = m.any(axis=(1, 3))   # [j, g]
    blk_all = m.all(axis=(1, 3))
    col_any = m.any(axis=3)        # [j, QC, g]
    tidx, mslot, offs, slots = [], [], [], {}
    for j in range(NJ):
        idxs, ms, os_ = [], [], []
        for i in range(LT):
            g0, g1 = 2 * i, 2 * i + 1
            if not (blk_any[j, g0] or blk_any[j, g1]):
                continue
            idxs.append(i)
            if blk_all[j, g0] and blk_all[j, g1]:
                ms.append(None)
                os_.append((0, 0))
            else:
                key = (mask[j * QC:(j + 1) * QC, g0 * P:(g0 + 1) * P].tobytes(),
                       mask[j * QC:(j + 1) * QC, g1 * P:(g1 + 1) * P].tobytes())
                ms.append(slots.setdefault(key, len(slots)))
                oo = []
                for g in (g0, g1):
                    nz = np.flatnonzero(col_any[j, :, g])
                    off = int(nz[0]) if len(nz) else QC
                    oo.append(off - off % 64)  # 64-elem align, conservative
                os_.append(tuple(oo))
        # first tile of a chunk is never trimmed: its start=True PV pass
        # must cover every output column of the accumulator
        if os_:
            os_[0] = (0, 0)
        tidx.append(tuple(idxs))
        mslot.append(tuple(ms))
        offs.append(tuple(os_))
    return tuple(tidx), tuple(mslot), tuple(offs), slots


def _mask_tables(mask, tidx, mslot, n_slots):
    """[2][n_slots, 128, 512] bf16 0/1 tiles (per parity)."""
    mp = [np.zeros((max(1, n_slots), P, QC), BF16_NP) for _ in range(2)]
    done = set()
    for j in range(NJ):
        for pos, i in enumerate(tidx[j]):
            s = mslot[j][pos]
            if s is None or s in done:
                continue
            done.add(s)
            for par in range(2):
                g = 2 * i + par
                blk = mask[j * QC:(j + 1) * QC, g * P:(g + 1) * P]
                mp[par][s] = blk.T.astype(BF16_NP)
    return mp


def _beat_blocks(tidx):
    """Assign attention beats (j, ii) to emission windows by data readiness;
    drains follow each chunk's last beat.  Falls back to chunk-sequential
    emission if the readiness-ordered schedule would need >3 concurrent
    PSUM accumulators."""
    ext = [len(t) for t in tidx]
    nbeats = [(e + 1) // 2 for e in ext]

    def win_of(j, ii):
        tiles = tidx[j][ii:ii + 2]
        return max(KVWIN[max(tiles) // (KVW // P)], QWIN[j])

    def entries_sorted():
        beats = []
        for j in range(NJ):
            for ii in range(0, ext[j], 2):
                beats.append((win_of(j, ii), j, ii))
        # within a window: finish already-open chunks first (frees their
        # PSUM accumulator before new chunks open), then alternate new
        # chunks ii-major so their S/exp/PV pipelines interleave
        fw = {}
        for w, j, ii in beats:
            fw[j] = min(fw.get(j, w), w)
        wmax = max(w for w, _, _ in beats)
        def grp(w, j):
            cont = fw[j] < w
            if w == wmax:      # final window: continuing chunks last keeps
                return 0 if not cont else 1   # the tail short
            return 0 if cont else 1
        beats.sort(key=lambda t: (t[0], grp(t[0], t[1]), t[2], t[1]))
        blocks = [[] for _ in range(NWIN)]
        seen = {j: 0 for j in range(NJ)}
        for w, j, ii in beats:
            blocks[w].append(("beat", j, ii))
            seen[j] += 1
            if seen[j] == nbeats[j]:
                blocks[w].append(("drain", j))
        for j in range(NJ):
            if ext[j] == 0:
                blocks[0].append(("zero", j))
                blocks[0].append(("drain", j))
        return blocks

    def ring_ok(blocks, ring=3):
        order = [e for b in blocks for e in b]
        open_order, drains = [], []
        for e in order:
            if e[0] in ("beat", "zero") and e[1] not in open_order:
                open_order.append(e[1])
                if len(open_order) > ring:
                    victim = open_order[len(open_order) - 1 - ring]
                    if victim not in drains:
                        return False
            elif e[0] == "drain":
                drains.append(e[1])
        return True

    blocks = entries_sorted()
    if ring_ok(blocks):
        return blocks
    # fallback: all beats of a chunk in the window where its last tile lands
    blocks = [[] for _ in range(NWIN)]
    for j in range(NJ):
        if ext[j] == 0:
            blocks[0] += [("zero", j), ("drain", j)]
            continue
        w = max(KVWIN[max(tidx[j]) // (KVW // P)], QWIN[j])
        for ii in range(0, ext[j], 2):
            blocks[w].append(("beat", j, ii))
        blocks[w].append(("drain", j))
    return blocks


def _build(tidx, mslot, offs, n_slots, par):
    n_mask = max(1, n_slots)
    nc = bacc.Bacc("TRN2", target_bir_lowering=False, debug=False,
                   num_devices=N_CORES)
    # q/k are staged fp8_e4m3: the PE accepts a bf16 stationary with an fp8
    # moving operand exactly (verified on HW), so this halves their HBM
    # traffic for ~1.1e-2 output error -- well inside the 2e-2 gate.
    qT = nc.dram_tensor("qT", [NJ, P, D_TILES, QC], FP8,
                        kind="ExternalInput")
    kT = nc.dram_tensor("kT", [NKV, P, D_TILES, KVW], FP8,
                        kind="ExternalInput")
    vT = nc.dram_tensor("vT", [NKV, P, D_TILES, KVW], BF16,
                        kind="ExternalInput")
    wk = nc.dram_tensor("wk", [P, D_TILES, 2 * H], BF16,
                        kind="ExternalInput")
    wqv = nc.dram_tensor("wqv", [P, D_TILES, 3 * H], BF16,
                         kind="ExternalInput")
    maskp = nc.dram_tensor("maskp", [P, n_mask, QC], BF16,
                           kind="ExternalInput")
    out = nc.dram_tensor("out", [NJ, H + 1, QC], BF16,
                     kind="ExternalOutput")

    Exp = mybir.ActivationFunctionType.Exp
    blocks = _beat_blocks(tidx)
    # drains emitted from round 3 onward can use the sync HWDGE queue: all
    # input DMAs have issued by then, so they can't stall the stream
    sync_drain_w = 2

    with tile.TileContext(nc) as tc:
        with (
            tc.tile_pool(name="const", bufs=1) as cpool,
            tc.tile_pool(name="proj", bufs=1) as projpool,
        ):
            wk_sb = cpool.tile([P, D_TILES, 2 * H], BF16)
            wqv_sb = cpool.tile([P, D_TILES, 3 * H], BF16)
            msk = cpool.tile([P, n_mask, QC], BF16)
            ident = cpool.tile([P, P], F32)
            warm = cpool.tile([P, 5 * P], BF16)

            qhT = projpool.tile([P, T], BF16, tag="qhT")      # qh in both halves
            khT = projpool.tile([P, LT // 2, P], BF16, tag="khT")
            vh = projpool.tile([P, LT, H + 1], BF16, tag="vh")

            with (
                tc.tile_pool(name="xs", bufs=12) as xpool,
                tc.tile_pool(name="xv", bufs=4) as xvpool,
                tc.tile_pool(name="pp", bufs=1, space="PSUM") as pppool,
                tc.tile_pool(name="sp", bufs=2, space="PSUM") as spool,
                tc.tile_pool(name="oac", bufs=3, space="PSUM") as opool,
                tc.tile_pool(name="vt", bufs=3) as vtpool,
                tc.tile_pool(name="pt", bufs=6) as ppool,
                tc.tile_pool(name="ost", bufs=2) as ostpool,
            ):
                oaccs = {}

                # ---------- emitter thunks ----------
                def dma_x(src, idx, dt, pool, tg):
                    def go():
                        xt = pool.tile([P, D_TILES, QC], dt, tag=tg,
                                       name="xt")
                        nc.sync.dma_start(out=xt[:], in_=src.ap()[idx])
                        return xt
                    return go

                def proj_thunks(xt_ref, wsb, wlo, whi, m_parts, out_cb,
                                width):
                    """8 matmul thunks accumulating [m_parts, width] then a
                    finisher callback on the psum tile."""
                    state = {}
                    def mk(dt_):
                        def go():
                            if dt_ == 0:
                                state["ps"] = pppool.tile(
                                    [m_parts, width], F32, tag="pp", name="ps")
                            nc.tensor.matmul(
                                state["ps"][:], lhsT=wsb[:, dt_, wlo:whi],
                                rhs=state["xt"][:, dt_, :],
                                start=(dt_ == 0), stop=(dt_ == D_TILES - 1))
                        return go
                    def first():
                        state["xt"] = xt_ref()
                    thunks = []
                    for dt_ in range(D_TILES):
                        if dt_ == 0:
                            g = mk(0)
                            thunks.append(lambda g=g: (first(), g()))
                        else:
                            thunks.append(mk(dt_))
                    thunks.append(lambda: out_cb(state["ps"]))
                    return thunks

                def q_finish(j):
                    def go(ps):
                        nc.vector.tensor_copy(
                            out=qhT[:, j * QC:(j + 1) * QC], in_=ps[:])
                    return go

                def k_finish(c):
                    def go(ps):
                        for t in range(2):
                            sl = 2 * c + t
                            nc.vector.tensor_copy(
                                out=khT[0:H, sl, :],
                                in_=ps[0:H, 2 * t * P:(2 * t + 1) * P])
                            nc.vector.tensor_copy(
                                out=khT[H:P, sl, :],
                                in_=ps[H:P, (2 * t + 1) * P:(2 * t + 2) * P])
                    return go

                def v_finish(c):
                    def go(ps):
                        vtmp = vtpool.tile([H + 1, KVW], F32, tag="vt",
                                           name="vtmp")
                        nc.vector.tensor_copy(out=vtmp[0:H, :], in_=ps[:])
                        nc.vector.memset(vtmp[H:H + 1, :], 1.0)
                        for tt in range(KVW // P):
                            tp = pppool.tile([P, H + 1], F32, tag="pp",
                                             name="tp")
                            nc.tensor.transpose(
                                tp[:], vtmp[:, tt * P:(tt + 1) * P],
                                ident[0:H + 1, 0:H + 1])
                            nc.vector.tensor_copy(
                                out=vh[:, c * (KVW // P) + tt, :], in_=tp[:])
                    return go

                def emit_beat(j, ii):
                    tiles = tidx[j][ii:ii + 2]
                    pw = len(tiles)
                    ext = len(tidx[j])
                    # packed valid spans: tile u covers query cols
                    # [off_u, QC) of the chunk, stored at [cur, cur+w_u)
                    offl = [offs[j][ii + u][par] for u in range(pw)]
                    ws = [QC - o for o in offl]
                    curs = [0]
                    for u in range(pw - 1):
                        curs.append(curs[u] + ws[u])
                    wtot = curs[-1] + ws[-1]
                    sp = spool.tile([P, 2 * QC], F32, tag="S", name="sp")
                    for u, i in enumerate(tiles):
                        half = (i % 2) * H
                        nc.tensor.matmul(
                            sp[:, curs[u]:curs[u] + ws[u]],
                            lhsT=khT[half:half + H, i // 2, :],
                            rhs=qhT[half:half + H,
                                    j * QC + offl[u]:(j + 1) * QC],
                            start=True, stop=True)
                    pt = ppool.tile([P, 2 * QC], BF16, tag="p", name="pt")
                    nc.scalar.activation(
                        out=pt[:, 0:wtot], in_=sp[:, 0:wtot],
                        func=Exp, scale=0.125)
                    for u in range(pw):
                        s = mslot[j][ii + u]
                        if s is not None:
                            nc.gpsimd.tensor_mul(
                                pt[:, curs[u]:curs[u] + ws[u]],
                                pt[:, curs[u]:curs[u] + ws[u]],
                                msk[:, s, offl[u]:QC])
                    def pv():
                        if ii == 0:
                            oaccs[j] = opool.tile([H + 1, QC], F32,
                                                  tag="oacc", name="oacc")
                        for u, i in enumerate(tiles):
                            nc.tensor.matmul(
                                oaccs[j][:, offl[u]:QC],
                                lhsT=vh[:, i, :],
                                rhs=pt[:, curs[u]:curs[u] + ws[u]],
                                start=(ii + u == 0),
                                stop=(ii + u == ext - 1))
                    return pv

                def emit_drain(j, use_sync):
                    ost = ostpool.tile([H + 1, QC], BF16, tag="ost",
                                       name="ost")
                    nc.vector.tensor_copy(out=ost[:], in_=oaccs[j][:])
                    if use_sync:
                        nc.sync.dma_start(out=out.ap()[j], in_=ost[:])
                    else:
                        # early drains go via the idle gpsimd queue: a
                        # sync-queue drain would stall later input DMA
                        # issue (the sync sequencer is in-order)
                        nc.gpsimd.dma_start(out=out.ap()[j], in_=ost[:])

                # ---------- emission ----------
                prev_beats = []       # beats of window r-1, emitted in round r
                for r in range(NWIN + 1):
                    # DMAs + projection thunks for this window's data
                    thunks = []
                    for ent in (WIN_DMA[r] if r < NWIN else []):
                        if ent[0] == "wk":
                            nc.sync.dma_start(out=wk_sb[:], in_=wk.ap())
                        elif ent[0] == "wqv":
                            nc.sync.dma_start(out=wqv_sb[:], in_=wqv.ap())
                        elif ent[0] == "masks":
                            nc.sync.dma_start(out=msk[:], in_=maskp.ap())
                        elif ent[0] == "k":
                            c = ent[1]
                            kx = dma_x(kT, c, FP8, xpool, "x")()
                            thunks += proj_thunks(
                                lambda kx=kx: kx, wk_sb, 0, 2 * H, P,
                                k_finish(c), KVW)
                        elif ent[0] == "v":
                            c = ent[1]
                            vx = dma_x(vT, c, BF16, xvpool, "xv")()
                            thunks += proj_thunks(
                                lambda vx=vx: vx, wqv_sb, 2 * H, 3 * H, H,
                                v_finish(c), KVW)
                        else:
                            j = ent[1]
                            qx = dma_x(qT, j, FP8, xpool, "x")()
                            thunks += proj_thunks(
                                lambda qx=qx: qx, wqv_sb, 0, 2 * H, P,
                                q_finish(j), QC)
                    if r == 0:
                        # all window-0 DMAs are now issued; warm the PE HAM
                        # clock gate on a memset tile while they stream
                        nc.vector.memset(warm[:], 0.125)
                        wps = pppool.tile([P, 4 * P], F32, tag="pp",
                                          name="wps")
                        for _ in range(N_WARM):
                            nc.tensor.matmul(
                                wps[:], lhsT=warm[:, 0:P],
                                rhs=warm[:, P:5 * P], start=True, stop=True)
                        make_identity(nc, ident[:])
                    # interleave previous window's beats with this one's proj
                    nb = max(1, len([e for e in prev_beats if e[0] == "beat"]))
                    skip = nb // 4   # this round's thunks wait on DMA still
                    ti = 0           # in flight; don't let them stall beats
                    bi = 0
                    for e in prev_beats:
                        if e[0] == "beat":
                            pv = emit_beat(e[1], e[2])
                            bi += 1
                            hi = (len(thunks) * max(0, bi - skip)
                                  // max(1, nb - skip))
                            while ti < hi:
                                thunks[ti]()
                                ti += 1
                            pv()
                        elif e[0] == "zero":
                            oaccs[e[1]] = opool.tile([H + 1, QC], F32,
                                                     tag="oacc", name="oacc")
                            nc.vector.memset(oaccs[e[1]][:], 0.0)
                        else:
                            emit_drain(e[1], r - 1 >= sync_drain_w)
                    while ti < len(thunks):
                        thunks[ti]()
                        ti += 1
                    prev_beats = blocks[r] if r < NWIN else []

    nc.compile()
    return nc


def _get_nc(key, tidx, mslot, offs, n_slots, par):
    if key not in _CACHE:
        _CACHE[key] = _build(tidx, mslot, offs, n_slots, par)
    return _CACHE[key]


def _tile_x(x2d, nchunks, width):
    """[D, nchunks*width] -> [nchunks, P, D_TILES, width] contiguous."""
    return np.ascontiguousarray(
        x2d.reshape(D_TILES, P, nchunks, width).transpose(2, 1, 0, 3))


def _make_in_maps(q, k, v, wkc, wqv, mp):
    cols = [np.concatenate(
        [np.arange((2 * i + par) * P, (2 * i + par + 1) * P)
         for i in range(LT)]) for par in range(2)]
    in_maps = []
    for c_ in range(N_CORES):
        b, par = divmod(c_, 2)
        qTb = _tile_x(q[b].T.astype(FP8_NP), NJ, QC)
        kTb = _tile_x(k[b].T[:, cols[par]].astype(FP8_NP), NKV, KVW)
        vTb = _tile_x(v[b].T[:, cols[par]].astype(BF16_NP), NKV, KVW)
        in_maps.append({
            "qT": qTb, "kT": kTb, "vT": vTb, "wk": wkc, "wqv": wqv,
            "maskp": np.ascontiguousarray(mp[par].transpose(1, 0, 2)),
        })
    return in_maps


def _gather_out(results):
    outp = np.empty((B, T, H), np.float32)
    for b in range(B):
        acc = (results[2 * b]["out"].astype(np.float32)
               + results[2 * b + 1]["out"].astype(np.float32))
        num = acc[:, 0:H, :]
        den = acc[:, H, :]
        outp[b] = (np.moveaxis(num, 1, 2) / den[:, :, None]).reshape(T, H)
    return outp


def kernel(q, k, v, Wq, Wk, Wv, attn_mask):
    global LAST_RESULT
    q = np.asarray(q, dtype=np.float32)
    k = np.asarray(k, dtype=np.float32)
    v = np.asarray(v, dtype=np.float32)
    mask = np.asarray(attn_mask).astype(bool)
    Wq = np.asarray(Wq, np.float32)
    Wk = np.asarray(Wk, np.float32)
    Wv = np.asarray(Wv, np.float32)
    # [Wk|Wk] / [Wq|Wq|Wv]: duplicated halves put qh/kh in both partition
    # halves; wk ships separately so the first k-projection starts sooner
    wkc = np.concatenate([Wk, Wk], axis=1).astype(BF16_NP)
    wkc = np.ascontiguousarray(
        wkc.reshape(D_TILES, P, 2 * H).transpose(1, 0, 2))
    wqv = np.concatenate([Wq, Wq, Wv], axis=1).astype(BF16_NP)
    wqv = np.ascontiguousarray(
        wqv.reshape(D_TILES, P, 3 * H).transpose(1, 0, 2))

    tidx, mslot, offs, slots = _schedule(mask)
    mp = _mask_tables(mask, tidx, mslot, len(slots))
    in_maps = _make_in_maps(q, k, v, wkc, wqv, mp)

    # the emitted program depends on the parity (trim offsets differ), but
    # all 8 cores must run ONE program: emit with the per-parity minimum
    # trim so both parities' matmuls cover their valid columns.
    offs_min = tuple(
        tuple((min(oo), min(oo)) for oo in offs[j]) for j in range(NJ))
    key = (tidx, mslot, offs_min, len(slots))
    nc = _get_nc(key, tidx, mslot, offs_min, len(slots), 0)

    res = run_bass_kernel_spmd(
        nc, in_maps, core_ids=list(range(N_CORES)),
        trace=bool(os.environ.get("KBENCH_TRACE")))
    LAST_RESULT = res
    return _gather_out(res.results)


# revision 13
# speedup vs baseline: 1.0717x; 1.0717x over previous
"""Distributed Trainium2 Bass kernel for a single attention head.

Problem (hardcoded): q,k,v [4, 4096, 1024] f32, Wq/Wk/Wv [1024, 64] f32,
attn_mask [4096, 4096] bool (True = keep).  out[b] = softmax(mask(q Wq (k Wk)^T) / 8) (v Wv).

Sharding: 8 cores; core c -> batch c//2, parity par = c%2.  The k/v rows of
the batch are split by 128-row k-tile parity: core par owns global k-tiles
{2i+par}.  Each core computes, for every 512-row query chunk j, the partial
(unnormalized) attention output sum_k exp(s)*v and the partial denominator
over ITS k-tiles only.  The host sums the two cores' partials and normalizes
(flash-attention style additive combine; no on-device collectives needed --
pair collectives have a ~7-20us latency floor, worse than the ~7us of PE
work they would dedup).

On-device layout / scheduling tricks (v2 -- tensor-engine-bound rework):
- All inputs are host-pre-tiled into the exact [128-partition, d-tile, col]
  SBUF layouts so every DMA is fully contiguous; every staged x chunk gets
  its own SBUF buffer so all input DMAs issue up front.
- q chunks are DMAed FIRST (q0..q3 in window 0) so projections (the only
  PE work available at the start) begin ~5.5us in, right after the ~3.4us
  engine-preamble barrier + first transfers; k/v stream behind.
- A burst of dummy matmuls on a memset tile warms the PE HAM clock gate
  (1.2 -> 2.4 GHz) during the otherwise-idle DMA ramp, so real matmuls
  start at the warm 2.4 GHz issue rate (214ns per 512-col matmul).
- Wq / Wk are duplicated column-wise in the packed weight so the projections
  produce qh / kh replicated in both partition halves.  Score matmuls have
  K=64; even/odd local k-tiles are stored in partition halves 0-63 / 64-127,
  so each beat's two score matmuls land in disjoint PE row-groups and
  overlap partially in the array (~385ns/pair vs 428ns serial).
- Causal diagonal trim: for score tiles whose leading query columns are
  fully masked, the score matmul streams only the valid column suffix and
  packs beats' valid spans contiguously in PSUM; exp() then runs on the
  packed (shorter) span and the PV matmul consumes the same packed slice,
  writing the matching suffix of the output accumulator.  The first tile of
  each chunk is never trimmed so the PV accumulation's start=True pass
  covers every output column.
- exp() runs on the packed beat span (<=1024 cols) to amortize the ~293ns
  ACT call overhead; mask multiplies for partially-kept 512x128 blocks run
  on the otherwise-idle GPSIMD engine (0/1 tiles from a tiny deduplicated
  table), keeping DVE free for the projection-finisher casts.
- Output partials are written as bf16 and drained via the gpsimd DMA queue
  while input DMAs are still issuing (the sync sequencer is in-order); the
  final window's drains go through the faster sync HWDGE queue instead,
  since by then all input DMAs have issued.
"""

import os
import sys

sys.path.insert(0, "/opt/trn_rl_repo")

import numpy as np
import ml_dtypes

import concourse.bass as bass
import concourse.mybir as mybir
import concourse.tile as tile
from concourse import bacc
from concourse.bass_utils import run_bass_kernel_spmd
from concourse.masks import make_identity

F32 = mybir.dt.float32
BF16 = mybir.dt.bfloat16
FP8 = mybir.dt.float8e4
BF16_NP = ml_dtypes.bfloat16
FP8_NP = ml_dtypes.float8_e4m3

N_CORES = 8
B, T, D, H = 4, 4096, 1024, 64
P = 128                      # partitions / k-tile rows
QC = 512                     # query chunk width
NJ = T // QC                 # 8 query chunks
GT = T // P                  # 32 global k-tiles
LT = GT // 2                 # 16 local (per-parity) k-tiles
D_TILES = D // P             # 8
KVW = 512                    # k/v projection chunk width (4 local tiles)
NKV = LT * P // KVW          # 4 kv chunks / emission blocks
# DMA window contents: q front-loaded so the PE has projection work as
# early as possible; k/v stream behind in tile order.
WIN_DMA = [
    [("wqv",), ("q", 0), ("wk",), ("k", 0), ("v", 0), ("q", 1), ("q", 2),
     ("q", 3)],
    [("masks",), ("q", 4), ("q", 5), ("k", 1), ("v", 1)],
    [("q", 6), ("q", 7), ("k", 2), ("v", 2)],
    [("k", 3), ("v", 3)],
    [],
]
NWIN = len(WIN_DMA)
KVWIN = [0, 1, 2, 3]             # window whose thunks project kv chunk c
QWIN = [0, 0, 0, 0, 1, 1, 2, 2]  # window whose thunks project q chunk j
N_WARM = 5                       # HAM warm-up matmuls

LAST_RESULT = None           # test harness reads exec_time_ns from here
_CACHE = {}


def _schedule(mask):
    """Per query chunk j: the list of local k-tile indices this parity pair
    processes (compile-time), per entry the mask-table slot to multiply
    with (None = block fully kept for both parities), and per entry the
    column trim offset (leading fully-masked query columns, same for both
    parities only if... computed per-parity at emission; here we store per
    (j, pos, par))."""
    m = mask.reshape(NJ, QC, GT, P)
    blk_any = m.any(axis=(1, 3))   # [j, g]
    blk_all = m.all(axis=(1, 3))
    col_any = m.any(axis=3)        # [j, QC, g]
    tidx, mslot, offs, slots = [], [], [], {}
    for j in range(NJ):
        idxs, ms, os_ = [], [], []
        for i in range(LT):
            g0, g1 = 2 * i, 2 * i + 1
            if not (blk_any[j, g0] or blk_any[j, g1]):
                continue
            idxs.append(i)
            if blk_all[j, g0] and blk_all[j, g1]:
                ms.append(None)
                os_.append((0, 0))
            else:
                key = (mask[j * QC:(j + 1) * QC, g0 * P:(g0 + 1) * P].tobytes(),
                       mask[j * QC:(j + 1) * QC, g1 * P:(g1 + 1) * P].tobytes())
                ms.append(slots.setdefault(key, len(slots)))
                oo = []
                for g in (g0, g1):
                    nz = np.flatnonzero(col_any[j, :, g])
                    off = int(nz[0]) if len(nz) else QC
                    oo.append(off - off % 64)  # 64-elem align, conservative
                os_.append(tuple(oo))
        # first tile of a chunk is never trimmed: its start=True PV pass
        # must cover every output column of the accumulator
        if os_:
            os_[0] = (0, 0)
        tidx.append(tuple(idxs))
        mslot.append(tuple(ms))
        offs.append(tuple(os_))
    return tuple(tidx), tuple(mslot), tuple(offs), slots


def _mask_tables(mask, tidx, mslot, n_slots):
    """[2][n_slots, 128, 512] bf16 0/1 tiles (per parity)."""
    mp = [np.zeros((max(1, n_slots), P, QC), BF16_NP) for _ in range(2)]
    done = set()
    for j in range(NJ):
        for pos, i in enumerate(tidx[j]):
            s = mslot[j][pos]
            if s is None or s in done:
                continue
            done.add(s)
            for par in range(2):
                g = 2 * i + par
                blk = mask[j * QC:(j + 1) * QC, g * P:(g + 1) * P]
                mp[par][s] = blk.T.astype(BF16_NP)
    return mp


def _beat_blocks(tidx):
    """Assign attention beats (j, ii) to emission windows by data readiness;
    drains follow each chunk's last beat.  Falls back to chunk-sequential
    emission if the readiness-ordered schedule would need >3 concurrent
    PSUM accumulators."""
    ext = [len(t) for t in tidx]
    nbeats = [(e + 1) // 2 for e in ext]

    def win_of(j, ii):
        tiles = tidx[j][ii:ii + 2]
        return max(KVWIN[max(tiles) // (KVW // P)], QWIN[j])

    def entries_sorted():
        beats = []
        for j in range(NJ):
            for ii in range(0, ext[j], 2):
                beats.append((win_of(j, ii), j, ii))
        # within a window: finish already-open chunks first (frees their
        # PSUM accumulator before new chunks open), then alternate new
        # chunks ii-major so their S/exp/PV pipelines interleave
        fw = {}
        for w, j, ii in beats:
            fw[j] = min(fw.get(j, w), w)
        wmax = max(w for w, _, _ in beats)
        def grp(w, j):
            cont = fw[j] < w
            if w == wmax:      # final window: continuing chunks last keeps
                return 0 if not cont else 1   # the tail short
            return 0 if cont else 1
        beats.sort(key=lambda t: (t[0], grp(t[0], t[1]), t[2], t[1]))
        blocks = [[] for _ in range(NWIN)]
        seen = {j: 0 for j in range(NJ)}
        for w, j, ii in beats:
            blocks[w].append(("beat", j, ii))
            seen[j] += 1
            if seen[j] == nbeats[j]:
                blocks[w].append(("drain", j))
        for j in range(NJ):
            if ext[j] == 0:
                blocks[0].append(("zero", j))
                blocks[0].append(("drain", j))
        return blocks

    def ring_ok(blocks, ring=3):
        order = [e for b in blocks for e in b]
        open_order, drains = [], []
        for e in order:
            if e[0] in ("beat", "zero") and e[1] not in open_order:
                open_order.append(e[1])
                if len(open_order) > ring:
                    victim = open_order[len(open_order) - 1 - ring]
                    if victim not in drains:
                        return False
            elif e[0] == "drain":
                drains.append(e[1])
        return True

    blocks = entries_sorted()
    if ring_ok(blocks):
        return blocks
    # fallback: all beats of a chunk in the window where its last tile lands
    blocks = [[] for _ in range(NWIN)]
    for j in range(NJ):
        if ext[j] == 0:
            blocks[0] += [("zero", j), ("drain", j)]
            continue
        w = max(KVWIN[max(tidx[j]) // (KVW // P)], QWIN[j])
        for ii in range(0, ext[j], 2):
            blocks[w].append(("beat", j, ii))
        blocks[w].append(("drain", j))
    return blocks


def _build(tidx, mslot, offs, n_slots, par):
    n_mask = max(1, n_slots)
    nc = bacc.Bacc("TRN2", target_bir_lowering=False, debug=False,
                   num_devices=N_CORES)
    # q/k are staged fp8_e4m3: the PE accepts a bf16 stationary with an fp8
    # moving operand exactly (verified on HW), so this halves their HBM
    # traffic for ~1.1e-2 output error -- well inside the 2e-2 gate.
    qT = nc.dram_tensor("qT", [NJ, P, D_TILES, QC], FP8,
                        kind="ExternalInput")
    kT = nc.dram_tensor("kT", [NKV, P, D_TILES, KVW], FP8,
                        kind="ExternalInput")
    vT = nc.dram_tensor("vT", [NKV, P, D_TILES, KVW], BF16,
                        kind="ExternalInput")
    wk = nc.dram_tensor("wk", [P, D_TILES, 2 * H], BF16,
                        kind="ExternalInput")
    wqv = nc.dram_tensor("wqv", [P, D_TILES, 3 * H], BF16,
                         kind="ExternalInput")
    maskp = nc.dram_tensor("maskp", [P, n_mask, QC], BF16,
                           kind="ExternalInput")
    out = nc.dram_tensor("out", [NJ, H + 1, QC], BF16,
                     kind="ExternalOutput")

    Exp = mybir.ActivationFunctionType.Exp
    blocks = _beat_blocks(tidx)
    # drains emitted from round 3 onward can use the sync HWDGE queue: all
    # input DMAs have issued by then, so they can't stall the stream
    sync_drain_w = 2

    with tile.TileContext(nc) as tc:
        with (
            tc.tile_pool(name="const", bufs=1) as cpool,
            tc.tile_pool(name="proj", bufs=1) as projpool,
        ):
            wk_sb = cpool.tile([P, D_TILES, 2 * H], BF16)
            wqv_sb = cpool.tile([P, D_TILES, 3 * H], BF16)
            msk = cpool.tile([P, n_mask, QC], BF16)
            ident = cpool.tile([P, P], F32)
            warm = cpool.tile([P, 5 * P], BF16)

            qhT = projpool.tile([P, T], BF16, tag="qhT")      # qh in both halves
            khT = projpool.tile([P, LT // 2, P], BF16, tag="khT")
            vh = projpool.tile([P, LT, H + 1], BF16, tag="vh")

            with (
                tc.tile_pool(name="xs", bufs=12) as xpool,
                tc.tile_pool(name="xv", bufs=4) as xvpool,
                tc.tile_pool(name="pp", bufs=1, space="PSUM") as pppool,
                tc.tile_pool(name="sp", bufs=2, space="PSUM") as spool,
                tc.tile_pool(name="oac", bufs=3, space="PSUM") as opool,
                tc.tile_pool(name="vt", bufs=3) as vtpool,
                tc.tile_pool(name="pt", bufs=6) as ppool,
                tc.tile_pool(name="ost", bufs=2) as ostpool,
            ):
                oaccs = {}

                # ---------- emitter thunks ----------
                def dma_x(src, idx, dt, pool, tg):
                    def go():
                        xt = pool.tile([P, D_TILES, QC], dt, tag=tg,
                                       name="xt")
                        nc.sync.dma_start(out=xt[:], in_=src.ap()[idx])
                        return xt
                    return go

                def proj_thunks(xt_ref, wsb, wlo, whi, m_parts, out_cb,
                                width):
                    """8 matmul thunks accumulating [m_parts, width] then a
                    finisher callback on the psum tile."""
                    state = {}
                    def mk(dt_):
                        def go():
                            if dt_ == 0:
                                state["ps"] = pppool.tile(
                                    [m_parts, width], F32, tag="pp", name="ps")
                            nc.tensor.matmul(
                                state["ps"][:], lhsT=wsb[:, dt_, wlo:whi],
                                rhs=state["xt"][:, dt_, :],
                                start=(dt_ == 0), stop=(dt_ == D_TILES - 1))
                        return go
                    def first():
                        state["xt"] = xt_ref()
                    thunks = []
                    for dt_ in range(D_TILES):
                        if dt_ == 0:
                            g = mk(0)
                            thunks.append(lambda g=g: (first(), g()))
                        else:
                            thunks.append(mk(dt_))
                    thunks.append(lambda: out_cb(state["ps"]))
                    return thunks

                def q_finish(j):
                    def go(ps):
                        nc.vector.tensor_copy(
                            out=qhT[:, j * QC:(j + 1) * QC], in_=ps[:])
                    return go

                def k_finish(c):
                    def go(ps):
                        for t in range(2):
                            sl = 2 * c + t
                            nc.vector.tensor_copy(
                                out=khT[0:H, sl, :],
                                in_=ps[0:H, 2 * t * P:(2 * t + 1) * P])
                            nc.vector.tensor_copy(
                                out=khT[H:P, sl, :],
                                in_=ps[H:P, (2 * t + 1) * P:(2 * t + 2) * P])
                    return go

                def v_finish(c):
                    def go(ps):
                        vtmp = vtpool.tile([H + 1, KVW], F32, tag="vt",
                                           name="vtmp")
                        nc.vector.tensor_copy(out=vtmp[0:H, :], in_=ps[:])
                        nc.vector.memset(vtmp[H:H + 1, :], 1.0)
                        for tt in range(KVW // P):
                            tp = pppool.tile([P, H + 1], F32, tag="pp",
                                             name="tp")
                            nc.tensor.transpose(
                                tp[:], vtmp[:, tt * P:(tt + 1) * P],
                                ident[0:H + 1, 0:H + 1])
                            nc.vector.tensor_copy(
                                out=vh[:, c * (KVW // P) + tt, :], in_=tp[:])
                    return go

                def emit_beat(j, ii, vec_mask=False):
                    tiles = tidx[j][ii:ii + 2]
                    pw = len(tiles)
                    ext = len(tidx[j])
                    # packed valid spans: tile u covers query cols
                    # [off_u, QC) of the chunk, stored at [cur, cur+w_u)
                    offl = [offs[j][ii + u][par] for u in range(pw)]
                    ws = [QC - o for o in offl]
                    curs = [0]
                    for u in range(pw - 1):
                        curs.append(curs[u] + ws[u])
                    wtot = curs[-1] + ws[-1]
                    if ii == 0:
                        oaccs[j] = opool.tile([H + 1, QC], F32,
                                              tag="oacc", name="oacc")
                    sp = spool.tile([P, 2 * QC], F32, tag="S", name="sp")
                    for u, i in enumerate(tiles):
                        half = (i % 2) * H
                        nc.tensor.matmul(
                            sp[:, curs[u]:curs[u] + ws[u]],
                            lhsT=khT[half:half + H, i // 2, :],
                            rhs=qhT[half:half + H,
                                    j * QC + offl[u]:(j + 1) * QC],
                            start=True, stop=True)
                    pt = ppool.tile([P, 2 * QC], BF16, tag="p", name="pt")
                    nc.scalar.activation(
                        out=pt[:, 0:wtot], in_=sp[:, 0:wtot],
                        func=Exp, scale=0.125)
                    for u in range(pw):
                        s = mslot[j][ii + u]
                        if s is not None:
                            # mask-mul on gpsimd keeps DVE free mid-kernel;
                            # the final window's beats use DVE (idle by
                            # then, and ~2x faster) to shorten the tail
                            eng = nc.vector if vec_mask else nc.gpsimd
                            eng.tensor_mul(
                                pt[:, curs[u]:curs[u] + ws[u]],
                                pt[:, curs[u]:curs[u] + ws[u]],
                                msk[:, s, offl[u]:QC])
                    def pv():
                        for u, i in enumerate(tiles):
                            nc.tensor.matmul(
                                oaccs[j][:, offl[u]:QC],
                                lhsT=vh[:, i, :],
                                rhs=pt[:, curs[u]:curs[u] + ws[u]],
                                start=(ii + u == 0),
                                stop=(ii + u == ext - 1))
                    return pv

                def emit_drain(j, use_sync):
                    ost = ostpool.tile([H + 1, QC], BF16, tag="ost",
                                       name="ost")
                    nc.vector.tensor_copy(out=ost[:], in_=oaccs[j][:])
                    if use_sync:
                        nc.sync.dma_start(out=out.ap()[j], in_=ost[:])
                    else:
                        # early drains go via the idle gpsimd queue: a
                        # sync-queue drain would stall later input DMA
                        # issue (the sync sequencer is in-order)
                        nc.gpsimd.dma_start(out=out.ap()[j], in_=ost[:])

                # ---------- emission ----------
                pending = []          # delayed PV closures [(j, pv)]
                prev_beats = []       # beats of window r-1, emitted in round r
                for r in range(NWIN + 1):
                    # DMAs + projection thunks for this window's data
                    thunks = []
                    for ent in (WIN_DMA[r] if r < NWIN else []):
                        if ent[0] == "wk":
                            nc.sync.dma_start(out=wk_sb[:], in_=wk.ap())
                        elif ent[0] == "wqv":
                            nc.sync.dma_start(out=wqv_sb[:], in_=wqv.ap())
                        elif ent[0] == "masks":
                            nc.sync.dma_start(out=msk[:], in_=maskp.ap())
                        elif ent[0] == "k":
                            c = ent[1]
                            kx = dma_x(kT, c, FP8, xpool, "x")()
                            thunks += proj_thunks(
                                lambda kx=kx: kx, wk_sb, 0, 2 * H, P,
                                k_finish(c), KVW)
                        elif ent[0] == "v":
                            c = ent[1]
                            vx = dma_x(vT, c, BF16, xvpool, "xv")()
                            thunks += proj_thunks(
                                lambda vx=vx: vx, wqv_sb, 2 * H, 3 * H, H,
                                v_finish(c), KVW)
                        else:
                            j = ent[1]
                            qx = dma_x(qT, j, FP8, xpool, "x")()
                            thunks += proj_thunks(
                                lambda qx=qx: qx, wqv_sb, 0, 2 * H, P,
                                q_finish(j), QC)
                    if r == 0:
                        # all window-0 DMAs are now issued; warm the PE HAM
                        # clock gate on a memset tile while they stream
                        nc.vector.memset(warm[:], 0.125)
                        wps = pppool.tile([P, 4 * P], F32, tag="pp",
                                          name="wps")
                        for _ in range(N_WARM):
                            nc.tensor.matmul(
                                wps[:], lhsT=warm[:, 0:P],
                                rhs=warm[:, P:5 * P], start=True, stop=True)
                        make_identity(nc, ident[:])
                    # interleave previous window's beats with this one's proj
                    nb = max(1, len([e for e in prev_beats if e[0] == "beat"]))
                    skip = nb // 4   # this round's thunks wait on DMA still
                    ti = 0           # in flight; don't let them stall beats
                    bi = 0
                    for e in prev_beats:
                        if e[0] == "beat":
                            pv = emit_beat(e[1], e[2],
                                           vec_mask=(r - 1 >= NWIN - 2))
                            pending.append((e[1], pv))
                            bi += 1
                            hi = (len(thunks) * max(0, bi - skip)
                                  // max(1, nb - skip))
                            while ti < hi:
                                thunks[ti]()
                                ti += 1
                            # delay each PV one beat behind its S/exp so the
                            # activation latency never stalls the PE queue
                            while len(pending) > 1:
                                pending.pop(0)[1]()
                        elif e[0] == "zero":
                            oaccs[e[1]] = opool.tile([H + 1, QC], F32,
                                                     tag="oacc", name="oacc")
                            nc.vector.memset(oaccs[e[1]][:], 0.0)
                        else:
                            jd = e[1]
                            keep = []
                            for jj, pv in pending:
                                if jj == jd:
                                    pv()
                                else:
                                    keep.append((jj, pv))
                            pending = keep
                            emit_drain(jd, r - 1 >= sync_drain_w)
                    while ti < len(thunks):
                        thunks[ti]()
                        ti += 1
                    prev_beats = blocks[r] if r < NWIN else []
                for _, pv in pending:
                    pv()

    nc.compile()
    return nc


def _get_nc(key, tidx, mslot, offs, n_slots, par):
    if key not in _CACHE:
        _CACHE[key] = _build(tidx, mslot, offs, n_slots, par)
    return _CACHE[key]


def _tile_x(x2d, nchunks, width):
    """[D, nchunks*width] -> [nchunks, P, D_TILES, width] contiguous."""
    return np.ascontiguousarray(
        x2d.reshape(D_TILES, P, nchunks, width).transpose(2, 1, 0, 3))


def _make_in_maps(q, k, v, wkc, wqv, mp):
    cols = [np.concatenate(
        [np.arange((2 * i + par) * P, (2 * i + par + 1) * P)
         for i in range(LT)]) for par in range(2)]
    in_maps = []
    for c_ in range(N_CORES):
        b, par = divmod(c_, 2)
        qTb = _tile_x(q[b].T.astype(FP8_NP), NJ, QC)
        kTb = _tile_x(k[b].T[:, cols[par]].astype(FP8_NP), NKV, KVW)
        vTb = _tile_x(v[b].T[:, cols[par]].astype(BF16_NP), NKV, KVW)
        in_maps.append({
            "qT": qTb, "kT": kTb, "vT": vTb, "wk": wkc, "wqv": wqv,
            "maskp": np.ascontiguousarray(mp[par].transpose(1, 0, 2)),
        })
    return in_maps


def _gather_out(results):
    outp = np.empty((B, T, H), np.float32)
    for b in range(B):
        acc = (results[2 * b]["out"].astype(np.float32)
               + results[2 * b + 1]["out"].astype(np.float32))
        num = acc[:, 0:H, :]
        den = acc[:, H, :]
        outp[b] = (np.moveaxis(num, 1, 2) / den[:, :, None]).reshape(T, H)
    return outp


def kernel(q, k, v, Wq, Wk, Wv, attn_mask):
    global LAST_RESULT
    q = np.asarray(q, dtype=np.float32)
    k = np.asarray(k, dtype=np.float32)
    v = np.asarray(v, dtype=np.float32)
    mask = np.asarray(attn_mask).astype(bool)
    Wq = np.asarray(Wq, np.float32)
    Wk = np.asarray(Wk, np.float32)
    Wv = np.asarray(Wv, np.float32)
    # [Wk|Wk] / [Wq|Wq|Wv]: duplicated halves put qh/kh in both partition
    # halves; wk ships separately so the first k-projection starts sooner
    wkc = np.concatenate([Wk, Wk], axis=1).astype(BF16_NP)
    wkc = np.ascontiguousarray(
        wkc.reshape(D_TILES, P, 2 * H).transpose(1, 0, 2))
    wqv = np.concatenate([Wq, Wq, Wv], axis=1).astype(BF16_NP)
    wqv = np.ascontiguousarray(
        wqv.reshape(D_TILES, P, 3 * H).transpose(1, 0, 2))

    tidx, mslot, offs, slots = _schedule(mask)
    mp = _mask_tables(mask, tidx, mslot, len(slots))
    in_maps = _make_in_maps(q, k, v, wkc, wqv, mp)

    # the emitted program depends on the parity (trim offsets differ), but
    # all 8 cores must run ONE program: emit with the per-parity minimum
    # trim so both parities' matmuls cover their valid columns.
    offs_min = tuple(
        tuple((min(oo), min(oo)) for oo in offs[j]) for j in range(NJ))
    key = (tidx, mslot, offs_min, len(slots))
    nc = _get_nc(key, tidx, mslot, offs_min, len(slots), 0)

    res = run_bass_kernel_spmd(
        nc, in_maps, core_ids=list(range(N_CORES)),
        trace=bool(os.environ.get("KBENCH_TRACE")))
    LAST_RESULT = res
    return _gather_out(res.results)
